# revision 11
# baseline (speedup 1.0000x reference)
"""Trainium2 Bass kernel for ArmLikenessGNN (GIN message passing + attention pooling).

v2 push-model architecture (8 NeuronCores, SPMD):
  - Shard by graph as before: core c owns graphs [8c, 8c+8), nodes laid out
    per-graph-padded (stride GP), so each core has NPAD nodes = NW windows of
    128. Node features kept transposed in SBUF ("T-layout") for the MLPs, and
    ALSO row-major bf16 (Hrow) for edge-selection matmuls.
  - Edge aggregation per layer:
      1. SELECTION (src core): edges are bucketed host-side into fixed cells
         (dst_core d, window w, src_block b) with K slots each. For each src
         block b, one-hot fp8 stationary matmuls select the edge source rows
         from Hrow into PSUM [slot, feat] tiles; flushed bf16 into a DRAM
         buffer G laid out [d][b][w][K] row-major.
      2. EXCHANGE: one AllToAll moves each d-region of G to its dst core,
         giving R = [src_core c][b][w][K].
      3. AGGREGATION (dst core): per window w, strided 1KB-run DMA loads the
         512 (c,b) cell runs into SBUF [cell, feat] tiles; one-hot fp8 moving
         matmuls accumulate agg^T[feat, dst] in PSUM.
  - Cell-overflow edges (count > K) fall back to the old pull path: bf16
    h-table AllGather + gpsimd.dma_gather + one-hot matmul tiles appended to
    the same per-window PSUM accumulation. This keeps the Q7 (SWDGE
    descriptor-generation) engine far off the critical path.
  - GIN MLP + LayerNorms computed in T-layout (cross-partition reductions via
    bf16 ones-matmuls; rsqrt fused on the scalar engine).
  - Attention pooling + head fully core-local; host concatenates 8x[8] logits.

All indexing (cell assignment, one-hot matrices, overflow gather lists) is
precomputed on host from the concrete inputs; the NEFF is compiled fresh
inside kernel() so every schedule constant is exact.
"""

import sys

sys.path.insert(0, "/opt/trn_rl_repo")

import numpy as np
import ml_dtypes

from concourse import bass, bacc, tile
from concourse import mybir
from concourse.bass_utils import run_bass_kernel_spmd

HID = 128
N_NODES = 50000
N_EDGES = 640000
N_GRAPHS = 64
N_LAYERS = 3
LN_EPS = 1e-5
NCORES = 8
GPC = N_GRAPHS // NCORES  # graphs per core
CH = 1024  # gather idxs per dma_gather call (overflow path)
NQUEUES = 1
W64 = 64  # padded window/block grid dimension
K = 4  # slots per (d, w, b) cell
T2 = (W64 * K) // 128  # PSUM tiles per (d, b) region (= K/2 for W64=64)
SPLIT = 32768  # int16 index ceiling -> two gather sides

F32 = mybir.dt.float32
BF16 = mybir.dt.bfloat16
FP8 = mybir.dt.float8e4
AF = mybir.ActivationFunctionType


# ----------------------------------------------------------------------------
# Host-side preprocessing
# ----------------------------------------------------------------------------

def _prep(x, edge_index, batch):
    """Compute the static layout, cell assignment and per-core data shards."""
    batch = np.asarray(batch).astype(np.int64)
    edge_index = np.asarray(edge_index).astype(np.int64)
    x = np.asarray(x).astype(np.float32)

    counts = np.bincount(batch, minlength=N_GRAPHS)
    starts = np.zeros(N_GRAPHS + 1, np.int64)
    np.cumsum(counts, out=starts[1:])
    GP = int(-(-(counts.max() + 1) // 128) * 128)  # graph stride, mult of 128
    NPAD = GPC * GP  # padded nodes per core
    NW = NPAD // 128  # windows (and src blocks) per core
    assert NW <= W64

    g_of = batch
    core_of = (g_of // GPC).astype(np.int64)
    col_of = (g_of % GPC) * GP + (np.arange(N_NODES) - starts[g_of])
    row_of = core_of * NPAD + col_of  # global table row

    src, dst = edge_index[0], edge_index[1]
    s_core = core_of[src]
    s_loc = row_of[src] - s_core * NPAD
    sB = s_loc // 128  # src block
    sb = s_loc % 128  # row within block
    d_core = core_of[dst]
    d_col = col_of[dst]
    w_of = d_col // 128
    dl = d_col % 128

    # ---- cell assignment: cell = (s_core, d_core, w, b), K slots each ----
    cell = ((s_core * NCORES + d_core) * W64 + w_of) * W64 + sB
    order = np.argsort(cell, kind="stable")
    cs = cell[order]
    new_grp = np.r_[True, cs[1:] != cs[:-1]]
    grp_starts_idx = np.flatnonzero(new_grp)
    grp_id = np.cumsum(new_grp) - 1
    rank = np.arange(len(cs)) - grp_starts_idx[grp_id]
    k_slot = rank
    assigned_s = k_slot < K
    e_sorted = order  # edge ids in cell-sorted order

    # structured (assigned) edges
    ae = e_sorted[assigned_s]
    ak = k_slot[assigned_s]
    a_score, a_dcore = s_core[ae], d_core[ae]
    a_w, a_b, a_sb, a_dl = w_of[ae], sB[ae], sb[ae], dl[ae]
    # psum mapping: region row r = w*K + k = p*T2 + t
    r_reg = a_w * K + ak
    a_p = r_reg // T2
    a_t = r_reg % T2

    NTB = NCORES * T2  # sel tiles per block
    # ohsel[core][sb][b*NTB + d*T2 + t][p]
    ohsel = np.zeros((NCORES, 128, NW * NTB, 128), np.float32)
    ohsel[a_score, a_sb, a_b * NTB + a_dcore * T2 + a_t, a_p] = 1.0
    # note: distinct edges never collide on (core, tile, p) by construction;
    # multiple edges CAN share the same sb row - but that's across different
    # (tile, p), so the fancy-index assignment above is collision-free.

    # ohagg[core][p][w*(4*K) + j*K + k][dl]; cell cb = c_src*W64 + b = 4p + j
    cb = a_score * W64 + a_b
    g_p = cb // 4
    g_j = cb % 4
    ohagg = np.zeros((NCORES, 128, NW * 4 * K, 128), np.float32)
    ohagg[a_dcore, g_p, a_w * (4 * K) + g_j * K + ak, a_dl] = 1.0
    # collisions: two edges in the same (dst core, w, j, k, p) would need the
    # same cell+slot -> impossible. Same (p, row) different dl: also distinct
    # edges -> distinct (cell, slot). Safe.

    # ---- overflow edges: old pull-path structures, per dst core ----
    oe = e_sorted[~assigned_s]
    o_dcore = d_core[oe]
    o_w = w_of[oe]
    o_dl = dl[oe]
    o_srow = row_of[src[oe]]

    cnt = np.zeros((NCORES, NW, 2), np.int64)
    per = [[[None, None] for _ in range(NW)] for _ in range(NCORES)]
    side_o = (o_srow >= SPLIT).astype(np.int64)
    key = (o_dcore * NW + o_w) * 2 + side_o
    sort2 = np.argsort(key, kind="stable")
    key_sorted = key[sort2]
    gs = np.searchsorted(key_sorted, np.arange(NCORES * NW * 2))
    ge = np.searchsorted(key_sorted, np.arange(NCORES * NW * 2), side="right")
    for c in range(NCORES):
        for w in range(NW):
            for s in range(2):
                kk = (c * NW + w) * 2 + s
                sel = sort2[gs[kk]:ge[kk]]
                per[c][w][s] = (o_srow[sel], o_dl[sel])
                cnt[c, w, s] = len(sel)

    T = np.maximum(0, -(-cnt.max(axis=0) // 128))  # [NW, 2] tiles
    nta = int(T[:, 0].sum())
    ntb = int(T[:, 1].sum())
    tpc = CH // 128
    nta_pad = max(tpc, -(-nta // tpc) * tpc)
    ntb_pad = max(tpc, -(-ntb // tpc) * tpc)
    SA, SB_ = nta_pad * 128, ntb_pad * 128

    idxA = np.zeros((NCORES, SA), np.int32)
    idxB = np.zeros((NCORES, SB_), np.int32)
    ohA = np.zeros((NCORES, SA, 128), np.float32)
    ohB = np.zeros((NCORES, SB_, 128), np.float32)
    for c in range(NCORES):
        for s, (idx_arr, oh_arr) in ((0, (idxA, ohA)), (1, (idxB, ohB))):
            off = 0
            for w in range(NW):
                rows, dloc = per[c][w][s]
                n = len(rows)
                idx_arr[c, off:off + n] = rows - (SPLIT if s else 0)
                oh_arr[c, off + np.arange(n), dloc] = 1.0
                off += int(T[w, s]) * 128
    del per

    def wrap16(a):  # logical i -> [i%16 rep x8, i//16]
        n = a.shape[-1]
        return np.tile(a.reshape(NCORES, n // 16, 16).transpose(0, 2, 1), (1, 8, 1))

    def wrap128(a):  # [C, S, 128] -> [C, 128, S/128, 128] tile-major
        S = a.shape[1]
        return np.ascontiguousarray(
            a.reshape(NCORES, S // 128, 128, 128).transpose(0, 2, 1, 3))

    idxA_w = wrap16(idxA).astype(np.int16)
    idxB_w = wrap16(idxB).astype(np.int16)
    ohA_w = wrap128(ohA).astype(ml_dtypes.float8_e4m3)
    ohB_w = wrap128(ohB).astype(ml_dtypes.float8_e4m3)

    # per-core x shards in T-layout + graph mask
    xo = np.zeros((NCORES, 13, NPAD), np.float32)
    xa = np.zeros((NCORES, 3, NPAD), np.float32)
    xr = np.zeros((NCORES, 3, NPAD), np.float32)
    gmask = np.full((NCORES, 1, NPAD), -1e30, np.float32)
    others = np.concatenate([x[:, :9], x[:, 15:19]], axis=1)  # [N,13]
    for c in range(NCORES):
        sel = np.nonzero(core_of == c)[0]
        cols = col_of[sel]
        xo[c][:, cols] = others[sel].T
        xa[c][:, cols] = x[sel, 9:12].T
        xr[c][:, cols] = x[sel, 12:15].T
        gmask[c, 0, cols] = 0.0

    layout = dict(GP=GP, NPAD=NPAD, NW=NW, SA=SA, SB=SB_, T=T, SPLIT=SPLIT,
                  nta_pad=nta_pad, ntb_pad=ntb_pad)
    shards = dict(xo=xo, xa=xa, xr=xr, gmask=gmask,
                  idxA=idxA_w, idxB=idxB_w, ohA=ohA_w, ohB=ohB_w,
                  ohsel=np.ascontiguousarray(
                      ohsel.transpose(0, 1, 2, 3)).astype(ml_dtypes.float8_e4m3),
                  ohagg=np.ascontiguousarray(
                      ohagg.transpose(0, 1, 2, 3)).astype(ml_dtypes.float8_e4m3))
    return layout, shards


def _prep_weights(P):
    """Transpose / fold weights into the layouts the kernel consumes."""
    w = {}

    def f32(a):
        return np.ascontiguousarray(np.asarray(a, np.float32))

    for p in ("axis", "org"):
        W1, b1 = f32(P[p + "_W1"]), f32(P[p + "_b1"])
        g, be = f32(P[p + "_g"]), f32(P[p + "_be"])
        W2, b2 = f32(P[p + "_W2"]), f32(P[p + "_b2"])
        w[p + "W1T"] = f32(W1.T)                       # [3,16]
        w[p + "b1"] = b1[:, None]                      # [16,1]
        w[p + "W2Tf"] = f32((W2 * g[None, :]).T)       # [16,16]
        w[p + "b2f"] = (W2 @ be + b2)[:, None]         # [16,1]
    ninW = f32(P["nin_W"])  # [128,45]
    w["ninWoT"] = f32(ninW[:, :13].T)   # [13,128]
    w["ninWaT"] = f32(ninW[:, 13:29].T)  # [16,128]
    w["ninWrT"] = f32(ninW[:, 29:45].T)  # [16,128]
    w["ninb"] = f32(P["nin_b"])[:, None]  # [128,1]
    for l in range(N_LAYERS):
        W1, b1 = f32(P["conv_W1"][l]), f32(P["conv_b1"][l])
        g, be = f32(P["conv_g"][l]), f32(P["conv_be"][l])
        W2, b2 = f32(P["conv_W2"][l]), f32(P["conv_b2"][l])
        w[f"W1T{l}"] = f32(W1.T)
        w[f"b1{l}"] = b1[:, None]
        w[f"W2Tf{l}"] = f32((W2 * g[None, :]).T)
        w[f"b2f{l}"] = (W2 @ be + b2)[:, None]
        w[f"lng{l}"] = f32(P["ln_g"][l])[:, None]
        w[f"lnb{l}"] = f32(P["ln_b"][l])[:, None]
        w[f"epsv{l}"] = np.full((128, 1), 1.0 + np.float32(P["eps"][l]), np.float32)
    for p, pre in (("gate", "g"), ("head", "h")):
        W1, b1 = f32(P[p + "_W1"]), f32(P[p + "_b1"])
        g, be = f32(P[p + "_g"]), f32(P[p + "_be"])
        W2, b2 = f32(P[p + "_W2"]), f32(P[p + "_b2"])
        w[pre + "W1T"] = f32(W1.T)                     # [128,64]
        w[pre + "b1"] = b1[:, None]                    # [64,1]
        w[pre + "W2Tf"] = f32((W2 * g[None, :]).T)     # [64,1]
        w[pre + "b2f"] = (W2 @ be + b2)[:, None]       # [1,1]
    return w


# column layout for the packed weight/constant tensor (shared host/builder)
def _wpack_layout():
    cols = {}
    off = 0

    def add(name, rows, ncols):
        nonlocal off
        cols[name] = (off, rows, ncols)
        off += ncols

    for p in ("axis", "org"):
        add(p + "W1T", 3, 16)
        add(p + "b1", 16, 1)
        add(p + "W2Tf", 16, 16)
        add(p + "b2f", 16, 1)
    add("ninWoT", 13, 128)
    add("ninWaT", 16, 128)
    add("ninWrT", 16, 128)
    add("ninb", 128, 1)
    for l in range(N_LAYERS):
        add(f"W1T{l}", 128, 128)
        add(f"b1{l}", 128, 1)
        add(f"W2Tf{l}", 128, 128)
        add(f"b2f{l}", 128, 1)
        add(f"lng{l}", 128, 1)
        add(f"lnb{l}", 128, 1)
        add(f"epsv{l}", 128, 1)
    for pre in ("g", "h"):
        add(pre + "W1T", 128, 64)
        add(pre + "b1", 64, 1)
        add(pre + "W2Tf", 64, 1)
        add(pre + "b2f", 1, 1)
    add("ident", 128, 128)
    add("inv_c1", 128, 1)
    add("inv16", 16, 1)
    add("inv64", 64, 1)
    add("ones_1r", 1, 128)
    add("epsc", 1, 1)
    return cols, off


def _pack_weights(P):
    w = _prep_weights(P)
    w["ident"] = np.eye(128, dtype=np.float32)
    w["inv_c1"] = np.full((128, 1), 1.0 / 128.0, np.float32)
    w["inv16"] = np.full((16, 1), 1.0 / 16.0, np.float32)
    w["inv64"] = np.full((64, 1), 1.0 / 64.0, np.float32)
    w["ones_1r"] = np.ones((1, 128), np.float32)
    w["epsc"] = np.full((1, 1), LN_EPS, np.float32)
    cols, total = _wpack_layout()
    pack = np.zeros((128, total), np.float32)
    for name, (off, rows, ncols) in cols.items():
        a = np.asarray(w[name], np.float32)
        assert a.shape == (rows, ncols), (name, a.shape, rows, ncols)
        pack[:rows, off:off + ncols] = a
    return pack


# ----------------------------------------------------------------------------
# Bass kernel builder
# ----------------------------------------------------------------------------

def _lnT(nc, pools, z_out, y, P, eps=None):
    """LayerNorm over the partition dim (P partitions, n cols), T-layout.
    z_out (sbuf) = (y - mean) * rsqrt(var + eps). y is sbuf [P, n].
    Cross-partition sums/broadcasts run as bf16 matmuls (4x PE rate)."""
    n = y.shape[-1]
    psb = pools["psB"]
    pss = pools["psS"]
    wtile = pools["wtile"]
    stile = pools["stile"]
    ones_c1b = pools["ones_c1b"]
    ones_1rb = pools["ones_1rb"]
    yb = wtile(P, n, BF16)
    nc.scalar.activation(yb[:], y[:], AF.Copy)
    mu = pss.tile([1, n], F32, tag="psS", name="mu")
    nc.tensor.matmul(mu[:], ones_c1b[:P, :], yb[:])            # [1,n] mean
    mu_sb = stile(n, BF16)
    nc.scalar.activation(mu_sb[:], mu[:], AF.Copy)
    mub = psb.tile([128, n], F32, tag="psB", name="mub")
    nc.tensor.matmul(mub[:P, :], ones_1rb[:1, :P], mu_sb[:])   # bcast [P,n]
    d = wtile(P, n)
    nc.vector.tensor_tensor(out=d[:], in0=y[:], in1=mub[:P, :],
                            op=mybir.AluOpType.subtract)
    sq = wtile(P, n, BF16)
    nc.scalar.activation(sq[:], d[:], AF.Square)
    v = pss.tile([1, n], F32, tag="psS", name="v")
    nc.tensor.matmul(v[:], ones_c1b[:P, :], sq[:])             # [1,n] var
    sd = stile(n)
    nc.scalar.activation(sd[:], v[:], AF.Sqrt, bias=pools["epsc"])
    ri_f = stile(n)
    nc.vector.reciprocal_approx_fast(ri_f[:], sd[:])
    ri = stile(n, BF16)
    nc.scalar.activation(ri[:], ri_f[:], AF.Copy)
    rb = psb.tile([128, n], F32, tag="psB", name="rb")
    nc.tensor.matmul(rb[:P, :], ones_1rb[:1, :P], ri[:])       # bcast
    nc.vector.tensor_tensor(out=z_out[:], in0=d[:], in1=rb[:P, :],
                            op=mybir.AluOpType.mult)


def build_kernel(layout):
    GP, NPAD, NW = layout["GP"], layout["NPAD"], layout["NW"]
    SA, SB_ = layout["SA"], layout["SB"]
    T = layout["T"]
    TPC = CH // 128
    TROWS = NCORES * NPAD
    SLAB = 512
    NSLAB = NPAD // SLAB
    WPS = SLAB // 128  # windows (and blocks) per slab
    NTB = NCORES * T2  # sel tiles per block
    REG = W64 * K      # rows per (d, b) region of G
    GROWS = NCORES * W64 * REG
    acum = np.concatenate([[0], np.cumsum(T[:, 0])]).astype(int)
    bcum = np.concatenate([[0], np.cumsum(T[:, 1])]).astype(int)

    nc = bacc.Bacc("TRN2", target_bir_lowering=False, debug=False,
                   num_devices=NCORES, dynamic_dma_scratch_size=49152,
                   num_swdge_queues=NQUEUES)

    def param(name, shape, dtype=F32):
        return nc.declare_dram_parameter(name, list(shape), dtype, isOutput=False)

    xo = param("xo", [13, NPAD])
    xa = param("xa", [3, NPAD])
    xr = param("xr", [3, NPAD])
    gmask = param("gmask", [1, NPAD])
    idxA = param("idxA", [128, SA // 16], mybir.dt.int16)
    idxB = param("idxB", [128, SB_ // 16], mybir.dt.int16)
    ohA = param("ohA", [128, SA // 128, 128], FP8)
    ohB = param("ohB", [128, SB_ // 128, 128], FP8)
    ohsel = param("ohsel", [128, NW * NTB, 128], FP8)
    ohagg = param("ohagg", [128, NW * 4 * K, 128], FP8)
    wcols, wtot = _wpack_layout()
    wpack = param("wpack", [128, wtot])
    out_ext = nc.declare_dram_parameter("out", [1, GPC], F32, isOutput=True)

    from contextlib import ExitStack
    with tile.TileContext(nc) as tc, ExitStack() as es:
            ep = es.enter_context
            constp = ep(tc.tile_pool(name="const", bufs=1))
            persist = ep(tc.tile_pool(name="persist", bufs=1))
            work = ep(tc.tile_pool(name="work", bufs=6))
            wxp = ep(tc.tile_pool(name="wxp", bufs=4))
            wsp = ep(tc.tile_pool(name="wsp", bufs=4))
            gatA = ep(tc.tile_pool(name="gatA", bufs=2))
            gatB = ep(tc.tile_pool(name="gatB", bufs=2))
            ohpA = ep(tc.tile_pool(name="ohpA", bufs=2))
            ohpB = ep(tc.tile_pool(name="ohpB", bufs=2))
            ohselp = ep(tc.tile_pool(name="ohselp", bufs=2))
            ohaggp = ep(tc.tile_pool(name="ohaggp", bufs=2))
            stgp = ep(tc.tile_pool(name="stgp", bufs=2))
            rwp = ep(tc.tile_pool(name="rwp", bufs=2))
            psB = ep(tc.tile_pool(name="psB", bufs=3, space="PSUM"))
            psS = ep(tc.tile_pool(name="psS", bufs=3, space="PSUM"))
            ps128 = ep(tc.tile_pool(name="ps128", bufs=2, space="PSUM"))
            dram = ep(tc.tile_pool(name="dram", bufs=1, space="DRAM"))
            def wtile(rows, ncols, dt=F32):
                return work.tile([128, 1024], dt, tag="w",
                                 name="wt")[:rows, :ncols]

            def stile(ncols, dt=F32):
                return wsp.tile([1, 1024], dt, tag="ws", name="st")[:, :ncols]

            # packed weights
            wp = constp.tile([128, wtot], F32, name="wp")
            nc.sync.dma_start(out=wp[:], in_=wpack[:])
            W = {}
            for nm, (off, rows, ncols) in wcols.items():
                W[nm] = wp[:rows, off:off + ncols]
            ident = W["ident"]
            ones_1r = W["ones_1r"]
            onesb = constp.tile([128, 132], BF16, name="onesb")
            nc.vector.memset(onesb[:], 0.0)
            nc.vector.tensor_copy(onesb[:, 0:1], W["inv_c1"])
            nc.vector.tensor_copy(onesb[:16, 1:2], W["inv16"])
            nc.vector.tensor_copy(onesb[:64, 2:3], W["inv64"])
            nc.vector.tensor_copy(onesb[:1, 4:132], W["ones_1r"])
            pools = dict(psB=psB, psS=psS, ps128=ps128, work=work,
                         epsc=W["epsc"], stile=stile, wtile=wtile,
                         ones_1rb=onesb[:1, 4:132])

            idxA_sb = persist.tile([128, SA // 16], mybir.dt.int16, name="idxAs")
            nc.sync.dma_start(out=idxA_sb[:], in_=idxA[:])
            idxB_sb = persist.tile([128, SB_ // 16], mybir.dt.int16, name="idxBs")
            nc.sync.dma_start(out=idxB_sb[:], in_=idxB[:])
            hT = persist.tile([128, NPAD], F32, name="hT")
            aggT = persist.tile([128, NPAD], BF16, name="aggT")
            Hrow = persist.tile([128, NW * 128], BF16, name="Hrow")
            G = persist.tile([128, GPC], F32, name="Gpool")  # pooled feats

            shard_bf = [dram.tile([NPAD, HID], BF16, tag=f"shard{l}",
                                  name=f"shard{l}") for l in range(N_LAYERS)]
            tables = [dram.tile([TROWS, HID], BF16, addr_space="Shared",
                                tag=f"table{l}", name=f"table{l}")
                      for l in range(N_LAYERS)]
            Gd = dram.tile([GROWS, HID], BF16, tag="Gd", name="Gd")
            Rd = dram.tile([GROWS, HID], BF16, tag="Rd", name="Rd")

            # rearranged views for scatter-flush and window loads
            def g_block_view(b):
                # [p (128), d (8), (t f) (T2*128)] for block b
                v = Gd[:, :].rearrange("(d b p t) f -> b p d (t f)",
                                       d=NCORES, b=W64, p=128, t=T2)
                return v[b]

            def r_window_view(w):
                # [cb (512), (k f) (K*128)] for window w
                v = Rd[:, :].rearrange("(q cb w k) f -> w q cb (k f)",
                                       q=4, cb=128, w=W64, k=K)
                return v[w]

            # zero-fill dummy-block regions of G once
            zstg = stgp.tile([128, NTB * 128], BF16, tag="stg", name="zstg")
            nc.vector.memset(zstg[:], 0.0)
            for b in range(NW, W64):
                nc.sync.dma_start(out=g_block_view(b), in_=zstg[:, :])

            def write_shard(l, s, src_sb):
                """Transpose hT slab s -> row-major bf16: Hrow + shard DRAM."""
                for kk in range(WPS):
                    b = s * WPS + kk
                    tp = ps128.tile([128, 128], F32, tag="ps128", name="tp")
                    nc.tensor.transpose(tp[:], src_sb[:, kk * 128:(kk + 1) * 128],
                                        ident)
                    bslab = Hrow[:, b * 128:(b + 1) * 128]
                    nc.scalar.activation(bslab, tp[:], AF.Copy)
                    nc.sync.dma_start(out=shard_bf[l][b * 128:(b + 1) * 128, :],
                                      in_=bslab)

            def select_block(b):
                """Edge-selection matmuls for src block b -> G region flush."""
                ohs = ohselp.tile([128, NTB, 128], FP8, tag="ohsel", name="ohs")
                nc.sync.dma_start(out=ohs[:],
                                  in_=ohsel[:, b * NTB:(b + 1) * NTB, :])
                stg = stgp.tile([128, NTB * 128], BF16, tag="stg", name="stg")
                hrb = Hrow[:, b * 128:(b + 1) * 128]
                for grp in range(NTB // 4):
                    ps = psB.tile([128, 512], F32, tag="psB", name="selp")
                    for j in range(4):
                        ti = grp * 4 + j
                        nc.tensor.matmul(ps[:, j * 128:(j + 1) * 128],
                                         ohs[:, ti, :], hrb)
                    if grp % 2 == 0:
                        nc.scalar.activation(
                            stg[:, grp * 512:(grp + 1) * 512], ps[:], AF.Copy)
                    else:
                        nc.vector.tensor_copy(
                            stg[:, grp * 512:(grp + 1) * 512], ps[:])
                nc.sync.dma_start(out=g_block_view(b), in_=stg[:, :])

            # =========================== encoder ===========================
            for s in range(NSLAB):
                c0, c1 = s * SLAB, (s + 1) * SLAB
                xo_sb = wxp.tile([13, SLAB], F32, tag="wx", name="xos")
                nc.sync.dma_start(out=xo_sb[:], in_=xo[:, c0:c1])
                xa_sb = wxp.tile([3, SLAB], F32, tag="wx", name="xas")
                nc.sync.dma_start(out=xa_sb[:], in_=xa[:, c0:c1])
                xr_sb = wxp.tile([3, SLAB], F32, tag="wx", name="xrs")
                nc.sync.dma_start(out=xr_sb[:], in_=xr[:, c0:c1])
                zz = {}
                for p, xin in (("axis", xa_sb), ("org", xr_sb)):
                    y1p = psB.tile([128, SLAB], F32, tag="psB", name="y1p")
                    nc.tensor.matmul(y1p[:16, :], W[p + "W1T"], xin[:])
                    y1 = wtile(16, SLAB)
                    nc.scalar.activation(y1[:], y1p[:16, :], AF.Relu,
                                         bias=W[p + "b1"])
                    z1 = wtile(16, SLAB)
                    pools["ones_c1b"] = onesb[:, 1:2]
                    _lnT(nc, pools, z1[:], y1[:], 16)
                    zp = psB.tile([128, SLAB], F32, tag="psB", name="zp")
                    nc.tensor.matmul(zp[:16, :], W[p + "W2Tf"], z1[:])
                    z2 = wtile(16, SLAB)
                    nc.vector.tensor_scalar_add(z2[:], zp[:16, :], W[p + "b2f"])
                    zz[p] = z2
                h0p = psB.tile([128, SLAB], F32, tag="psB", name="h0p")
                nc.tensor.matmul(h0p[:], W["ninWoT"], xo_sb[:],
                                 start=True, stop=False)
                nc.tensor.matmul(h0p[:], W["ninWaT"], zz["axis"][:],
                                 start=False, stop=False)
                nc.tensor.matmul(h0p[:], W["ninWrT"], zz["org"][:],
                                 start=False, stop=True)
                nc.vector.tensor_scalar_add(hT[:, c0:c1], h0p[:], W["ninb"])
                write_shard(0, s, hT[:, c0:c1])
                for kk in range(WPS):
                    select_block(s * WPS + kk)

            # exchange + table gather for layer 0
            nc.gpsimd.collective_compute(
                "AllToAll", mybir.AluOpType.bypass,
                replica_groups=[list(range(NCORES))],
                ins=[Gd[:, :]], outs=[Rd[:, :]],
            )
            nc.gpsimd.collective_compute(
                "AllGather", mybir.AluOpType.bypass,
                replica_groups=[list(range(NCORES))],
                ins=[shard_bf[0].opt()], outs=[tables[0].opt()],
            )

            # =========================== GIN layers ===========================
            qsel = [0]
            for l in range(N_LAYERS):
                side_cfg = (
                    (idxA_sb, ohA, tables[l][0:SPLIT, :], gatA, ohpA),
                    (idxB_sb, ohB, tables[l][SPLIT:TROWS, :], gatB, ohpB),
                )
                chunk_tiles = [{}, {}]

                def get_chunk(sd, c):
                    if c not in chunk_tiles[sd]:
                        idx_sb, oh_dram, tbl_s, gp, op = side_cfg[sd]
                        g = gp.tile([128, TPC, HID], BF16, tag=f"gat{sd}",
                                    name=f"g{sd}")
                        nc.gpsimd.dma_gather(
                            out_ap=g[:],
                            in_ap=tbl_s[:, :],
                            idxs_ap=idx_sb[:, c * (CH // 16):(c + 1) * (CH // 16)],
                            num_idxs=CH,
                            num_idxs_reg=CH,
                            elem_size=HID,
                            queue_num=qsel[0] % NQUEUES,
                        )
                        qsel[0] += 1
                        oh = op.tile([128, TPC, 128], FP8, tag=f"oh{sd}",
                                     name=f"oh{sd}")
                        nc.sync.dma_start(
                            out=oh[:],
                            in_=oh_dram[:, c * TPC:(c + 1) * TPC, :])
                        chunk_tiles[sd][c] = (g, oh)
                    return chunk_tiles[sd][c]

                for s in range(NSLAB):
                    for w in range(s * WPS, (s + 1) * WPS):
                        # structured cells
                        rw = rwp.tile([128, 4 * K * 128], BF16, tag="rw",
                                      name="rw")
                        nc.sync.dma_start(out=rw[:, :], in_=r_window_view(w))
                        oha = ohaggp.tile([128, 4 * K, 128], FP8, tag="ohagg",
                                          name="oha")
                        nc.sync.dma_start(
                            out=oha[:],
                            in_=ohagg[:, w * 4 * K:(w + 1) * 4 * K, :])
                        seq = ([(0, acum[w] + j) for j in range(T[w, 0])]
                               + [(1, bcum[w] + j) for j in range(T[w, 1])])
                        psag = ps128.tile([128, 128], F32, tag="ps128",
                                          name="psag")
                        nmm = 4 * K + len(seq)
                        i = 0
                        for jk in range(4 * K):
                            nc.tensor.matmul(
                                psag[:],
                                rw[:, jk * 128:(jk + 1) * 128],
                                oha[:, jk, :],
                                start=(i == 0), stop=(i == nmm - 1))
                            i += 1
                        for (sd, gt) in seq:
                            g, oh = get_chunk(sd, gt // TPC)
                            t = gt % TPC
                            nc.tensor.matmul(psag[:], g[:, t, :], oh[:, t, :],
                                             start=(i == 0),
                                             stop=(i == nmm - 1))
                            i += 1
                        nc.vector.tensor_copy(aggT[:, w * 128:(w + 1) * 128],
                                              psag[:])

                    c0, c1 = s * SLAB, (s + 1) * SLAB
                    m = wtile(128, SLAB)
                    nc.vector.scalar_tensor_tensor(
                        out=m[:], in0=hT[:, c0:c1], scalar=W[f"epsv{l}"],
                        in1=aggT[:, c0:c1], op0=mybir.AluOpType.mult,
                        op1=mybir.AluOpType.add)
                    y1p = psB.tile([128, SLAB], F32, tag="psB", name="y1p")
                    nc.tensor.matmul(y1p[:], W[f"W1T{l}"], m[:])
                    y1 = wtile(128, SLAB)
                    nc.scalar.activation(y1[:], y1p[:], AF.Relu, bias=W[f"b1{l}"])
                    z1 = wtile(128, SLAB)
                    pools["ones_c1b"] = onesb[:, 0:1]
                    _lnT(nc, pools, z1[:], y1[:], 128)
                    y2p = psB.tile([128, SLAB], F32, tag="psB", name="y2p")
                    nc.tensor.matmul(y2p[:], W[f"W2Tf{l}"], z1[:])
                    r2 = wtile(128, SLAB)
                    nc.scalar.activation(r2[:], y2p[:], AF.Relu, bias=W[f"b2f{l}"])
                    z2 = wtile(128, SLAB)
                    pools["ones_c1b"] = onesb[:, 0:1]
                    _lnT(nc, pools, z2[:], r2[:], 128)
                    zs = wtile(128, SLAB)
                    nc.vector.tensor_scalar(out=zs[:], in0=z2[:],
                                            scalar1=W[f"lng{l}"],
                                            op0=mybir.AluOpType.mult,
                                            scalar2=W[f"lnb{l}"],
                                            op1=mybir.AluOpType.add)
                    nc.vector.tensor_tensor(out=hT[:, c0:c1], in0=zs[:],
                                            in1=hT[:, c0:c1],
                                            op=mybir.AluOpType.add)
                    if l + 1 < N_LAYERS:
                        write_shard(l + 1, s, hT[:, c0:c1])
                        for kk in range(WPS):
                            select_block(s * WPS + kk)

                if l + 1 < N_LAYERS:
                    nc.gpsimd.collective_compute(
                        "AllToAll", mybir.AluOpType.bypass,
                        replica_groups=[list(range(NCORES))],
                        ins=[Gd[:, :]], outs=[Rd[:, :]],
                    )
                    nc.gpsimd.collective_compute(
                        "AllGather", mybir.AluOpType.bypass,
                        replica_groups=[list(range(NCORES))],
                        ins=[shard_bf[l + 1].opt()], outs=[tables[l + 1].opt()],
                    )

            # ===================== pooling + head =====================
            gate_d = dram.tile([1, NPAD], F32, tag="gate_d", name="gate_d")
            for s in range(NSLAB):
                c0, c1 = s * SLAB, (s + 1) * SLAB
                y1p = psB.tile([128, SLAB], F32, tag="psB", name="y1p")
                nc.tensor.matmul(y1p[:64, :], W["gW1T"], hT[:, c0:c1])
                y1 = wtile(64, SLAB)
                nc.scalar.activation(y1[:], y1p[:64, :], AF.Relu, bias=W["gb1"])
                z1 = wtile(64, SLAB)
                pools["ones_c1b"] = onesb[:, 2:3]
                _lnT(nc, pools, z1[:], y1[:], 64)
                gp = psS.tile([1, SLAB], F32, tag="psS", name="gp")
                nc.tensor.matmul(gp[:], W["gW2Tf"], z1[:])
                gsb = stile(SLAB)
                nc.vector.tensor_scalar_add(gsb[:], gp[:], W["gb2f"])
                gm_sb = stile(SLAB)
                nc.sync.dma_start(out=gm_sb[:], in_=gmask[:, c0:c1])
                gfin = stile(SLAB)
                nc.vector.tensor_tensor(out=gfin[:], in0=gsb[:], in1=gm_sb[:],
                                        op=mybir.AluOpType.add)
                nc.sync.dma_start(out=gate_d[:, c0:c1], in_=gfin[:])

            for gi in range(GPC):
                c0, c1 = gi * GP, (gi + 1) * GP
                gate_sb = stile(GP)
                nc.sync.dma_start(out=gate_sb[:], in_=gate_d[:, c0:c1])
                gmax = stile(1)
                nc.vector.tensor_reduce(out=gmax[:], in_=gate_sb[:],
                                        axis=mybir.AxisListType.X,
                                        op=mybir.AluOpType.max)
                negmax = stile(1)
                nc.vector.tensor_scalar_mul(negmax[:], gmax[:], -1.0)
                e = stile(GP)
                nc.scalar.activation(e[:], gate_sb[:], AF.Exp, bias=negmax[:])
                den = stile(1)
                nc.vector.tensor_reduce(out=den[:], in_=e[:],
                                        axis=mybir.AxisListType.X,
                                        op=mybir.AluOpType.add)
                rden = stile(1)
                nc.vector.reciprocal(rden[:], den[:])
                attn = stile(GP)
                nc.vector.tensor_scalar_mul(attn[:], e[:], rden[:])
                half = GP // 2
                gacc = wtile(128, 2)
                for hf in range(2):
                    h0, h1 = c0 + hf * half, c0 + (hf + 1) * half
                    ab = psB.tile([128, half], F32, tag="psB", name="ab")
                    nc.tensor.matmul(ab[:], ones_1r,
                                     attn[:, hf * half:(hf + 1) * half])
                    wh = wtile(128, half)
                    nc.vector.tensor_tensor(out=wh[:], in0=hT[:, h0:h1],
                                            in1=ab[:], op=mybir.AluOpType.mult)
                    nc.vector.tensor_reduce(out=gacc[:, hf:hf + 1], in_=wh[:],
                                            axis=mybir.AxisListType.X,
                                            op=mybir.AluOpType.add)
                nc.vector.tensor_reduce(out=G[:, gi:gi + 1], in_=gacc[:],
                                        axis=mybir.AxisListType.X,
                                        op=mybir.AluOpType.add)

            y1p = psB.tile([128, GPC], F32, tag="psB", name="y1ph")
            nc.tensor.matmul(y1p[:64, :], W["hW1T"], G[:])
            y1 = wtile(64, GPC)
            nc.scalar.activation(y1[:], y1p[:64, :], AF.Relu, bias=W["hb1"])
            z1 = wtile(64, GPC)
            pools["ones_c1b"] = onesb[:, 2:3]
            _lnT(nc, pools, z1[:], y1[:], 64)
            lp = psS.tile([1, GPC], F32, tag="psS", name="lp")
            nc.tensor.matmul(lp[:], W["hW2Tf"], z1[:])
            logit = stile(GPC)
            nc.vector.tensor_scalar_add(logit[:], lp[:], W["hb2f"])
            nc.sync.dma_start(out=out_ext[:], in_=logit[:])

    nc.compile()
    return nc


# ----------------------------------------------------------------------------
# Public entry point
# ----------------------------------------------------------------------------

_CACHE = {}


def kernel(**inputs):
    x = inputs["x"]
    edge_index = inputs["edge_index"]
    batch = inputs["batch"]

    layout, shards = _prep(x, edge_index, batch)
    wpack = _pack_weights(inputs)

    key = (layout["GP"], layout["SA"], layout["SB"],
           layout["T"].tobytes())
    nc = _CACHE.get(key)
    if nc is None:
        nc = build_kernel(layout)
        _CACHE[key] = nc

    in_maps = []
    for c in range(NCORES):
        m = {
            "xo": shards["xo"][c], "xa": shards["xa"][c], "xr": shards["xr"][c],
            "gmask": shards["gmask"][c],
            "idxA": shards["idxA"][c], "idxB": shards["idxB"][c],
            "ohA": shards["ohA"][c], "ohB": shards["ohB"][c],
            "ohsel": shards["ohsel"][c], "ohagg": shards["ohagg"][c],
            "wpack": wpack,
        }
        in_maps.append(m)

    import os
    trace = os.environ.get("GNN_KERNEL_TRACE", "1") == "1"
    try:
        res = run_bass_kernel_spmd(nc, in_maps, core_ids=list(range(NCORES)),
                                   trace=trace)
    except Exception:
        if not trace:
            raise
        res = run_bass_kernel_spmd(nc, in_maps, core_ids=list(range(NCORES)),
                                   trace=False)
    global LAST_EXEC_NS
    LAST_EXEC_NS = res.exec_time_ns
    out = np.concatenate([np.asarray(res.results[c]["out"]).reshape(-1)
                          for c in range(NCORES)])
    return out.astype(np.float32)


LAST_EXEC_NS = None


if __name__ == "__main__":
    sys.path.insert(0, "/root/problem")
    import reference
    inp = reference.setup_inputs()
    got = kernel(**{k: np.asarray(v) for k, v in inp.items()})
    exp = np.asarray(reference.reference(**inp))
    err = np.linalg.norm(got - exp) / np.linalg.norm(exp)
    print("Relative error:", err)


# revision 18
# speedup vs baseline: 1.2172x; 1.2172x over previous
"""Trainium2 Bass kernel for ArmLikenessGNN (GIN message passing + attention pooling).

v2 push-model architecture (8 NeuronCores, SPMD):
  - Shard by graph as before: core c owns graphs [8c, 8c+8), nodes laid out
    per-graph-padded (stride GP), so each core has NPAD nodes = NW windows of
    128. Node features kept transposed in SBUF ("T-layout") for the MLPs, and
    ALSO row-major bf16 (Hrow) for edge-selection matmuls.
  - Edge aggregation per layer:
      1. SELECTION (src core): edges are bucketed host-side into fixed cells
         (dst_core d, window w, src_block b) with K slots each. For each src
         block b, one-hot fp8 stationary matmuls select the edge source rows
         from Hrow into PSUM [slot, feat] tiles; flushed bf16 into a DRAM
         buffer G laid out [d][b][w][K] row-major.
      2. EXCHANGE: one AllToAll moves each d-region of G to its dst core,
         giving R = [src_core c][b][w][K].
      3. AGGREGATION (dst core): per window w, strided 1KB-run DMA loads the
         512 (c,b) cell runs into SBUF [cell, feat] tiles; one-hot fp8 moving
         matmuls accumulate agg^T[feat, dst] in PSUM.
  - Cell-overflow edges (count > K) fall back to the old pull path: bf16
    h-table AllGather + gpsimd.dma_gather + one-hot matmul tiles appended to
    the same per-window PSUM accumulation. This keeps the Q7 (SWDGE
    descriptor-generation) engine far off the critical path.
  - GIN MLP + LayerNorms computed in T-layout (cross-partition reductions via
    bf16 ones-matmuls; rsqrt fused on the scalar engine).
  - Attention pooling + head fully core-local; host concatenates 8x[8] logits.

All indexing (cell assignment, one-hot matrices, overflow gather lists) is
precomputed on host from the concrete inputs; the NEFF is compiled fresh
inside kernel() so every schedule constant is exact.
"""

import sys

sys.path.insert(0, "/opt/trn_rl_repo")

import numpy as np
import ml_dtypes

from concourse import bass, bacc, tile
from concourse import mybir
from concourse.bass_utils import run_bass_kernel_spmd

HID = 128
N_NODES = 50000
N_EDGES = 640000
N_GRAPHS = 64
N_LAYERS = 3
LN_EPS = 1e-5
NCORES = 8
GPC = N_GRAPHS // NCORES  # graphs per core
CH = 1024  # gather idxs per dma_gather call (overflow path)
NQUEUES = 1
W64 = 64  # padded window/block grid dimension
K = 2  # slots per (d, w, b) cell
T2 = (W64 * K) // 128  # PSUM tiles per (d, b) region (= K/2 for W64=64)
SPLIT = 32768  # int16 index ceiling -> two gather sides

F32 = mybir.dt.float32
BF16 = mybir.dt.bfloat16
FP8 = mybir.dt.float8e4
AF = mybir.ActivationFunctionType


# ----------------------------------------------------------------------------
# Host-side preprocessing
# ----------------------------------------------------------------------------

def _prep(x, edge_index, batch):
    """Compute the static layout, cell assignment and per-core data shards."""
    batch = np.asarray(batch).astype(np.int64)
    edge_index = np.asarray(edge_index).astype(np.int64)
    x = np.asarray(x).astype(np.float32)

    counts = np.bincount(batch, minlength=N_GRAPHS)
    starts = np.zeros(N_GRAPHS + 1, np.int64)
    np.cumsum(counts, out=starts[1:])
    GP = int(-(-(counts.max() + 1) // 128) * 128)  # graph stride, mult of 128
    NPAD = GPC * GP  # padded nodes per core
    NW = NPAD // 128  # windows (and src blocks) per core
    assert NW <= W64

    g_of = batch
    core_of = (g_of // GPC).astype(np.int64)
    col_of = (g_of % GPC) * GP + (np.arange(N_NODES) - starts[g_of])
    row_of = core_of * NPAD + col_of  # global table row

    src, dst = edge_index[0], edge_index[1]
    s_core = core_of[src]
    s_loc = row_of[src] - s_core * NPAD
    sB = s_loc // 128  # src block
    sb = s_loc % 128  # row within block
    d_core = core_of[dst]
    d_col = col_of[dst]
    w_of = d_col // 128
    dl = d_col % 128

    # ---- cell assignment: cell = (s_core, d_core, w, b), K slots each ----
    cell = ((s_core * NCORES + d_core) * W64 + w_of) * W64 + sB
    order = np.argsort(cell, kind="stable")
    cs = cell[order]
    new_grp = np.r_[True, cs[1:] != cs[:-1]]
    grp_starts_idx = np.flatnonzero(new_grp)
    grp_id = np.cumsum(new_grp) - 1
    rank = np.arange(len(cs)) - grp_starts_idx[grp_id]
    k_slot = rank
    assigned_s = k_slot < K
    e_sorted = order  # edge ids in cell-sorted order

    # structured (assigned) edges
    ae = e_sorted[assigned_s]
    ak = k_slot[assigned_s]
    a_score, a_dcore = s_core[ae], d_core[ae]
    a_w, a_b, a_sb, a_dl = w_of[ae], sB[ae], sb[ae], dl[ae]
    # psum mapping: region row r = w*K + k = p*T2 + t
    r_reg = a_w * K + ak
    a_p = r_reg // T2
    a_t = r_reg % T2

    NTB = NCORES * T2  # sel tiles per block
    # ohsel[core][sb][b*NTB + d*T2 + t][p]
    ohsel = np.zeros((NCORES, 128, NW * NTB, 128), np.float32)
    ohsel[a_score, a_sb, a_b * NTB + a_dcore * T2 + a_t, a_p] = 1.0
    # note: distinct edges never collide on (core, tile, p) by construction;
    # multiple edges CAN share the same sb row - but that's across different
    # (tile, p), so the fancy-index assignment above is collision-free.

    # ohagg[core][p][w*(4*K) + j*K + k][dl]; cell cb = c_src*W64 + b = 4p + j
    cb = a_score * W64 + a_b
    g_p = cb // 4
    g_j = cb % 4
    ohagg = np.zeros((NCORES, 128, NW * 4 * K, 128), np.float32)
    ohagg[a_dcore, g_p, a_w * (4 * K) + g_j * K + ak, a_dl] = 1.0
    # collisions: two edges in the same (dst core, w, j, k, p) would need the
    # same cell+slot -> impossible. Same (p, row) different dl: also distinct
    # edges -> distinct (cell, slot). Safe.

    # ---- overflow edges: old pull-path structures, per dst core ----
    oe = e_sorted[~assigned_s]
    o_dcore = d_core[oe]
    o_w = w_of[oe]
    o_dl = dl[oe]
    o_srow = row_of[src[oe]]

    cnt = np.zeros((NCORES, NW, 2), np.int64)
    per = [[[None, None] for _ in range(NW)] for _ in range(NCORES)]
    side_o = (o_srow >= SPLIT).astype(np.int64)
    key = (o_dcore * NW + o_w) * 2 + side_o
    sort2 = np.argsort(key, kind="stable")
    key_sorted = key[sort2]
    gs = np.searchsorted(key_sorted, np.arange(NCORES * NW * 2))
    ge = np.searchsorted(key_sorted, np.arange(NCORES * NW * 2), side="right")
    for c in range(NCORES):
        for w in range(NW):
            for s in range(2):
                kk = (c * NW + w) * 2 + s
                sel = sort2[gs[kk]:ge[kk]]
                per[c][w][s] = (o_srow[sel], o_dl[sel])
                cnt[c, w, s] = len(sel)

    T = np.maximum(0, -(-cnt.max(axis=0) // 128))  # [NW, 2] tiles
    nta = int(T[:, 0].sum())
    ntb = int(T[:, 1].sum())
    tpc = CH // 128
    nta_pad = max(tpc, -(-nta // tpc) * tpc)
    ntb_pad = max(tpc, -(-ntb // tpc) * tpc)
    SA, SB_ = nta_pad * 128, ntb_pad * 128

    idxA = np.zeros((NCORES, SA), np.int32)
    idxB = np.zeros((NCORES, SB_), np.int32)
    ohA = np.zeros((NCORES, SA, 128), np.float32)
    ohB = np.zeros((NCORES, SB_, 128), np.float32)
    for c in range(NCORES):
        for s, (idx_arr, oh_arr) in ((0, (idxA, ohA)), (1, (idxB, ohB))):
            off = 0
            for w in range(NW):
                rows, dloc = per[c][w][s]
                n = len(rows)
                idx_arr[c, off:off + n] = rows - (SPLIT if s else 0)
                oh_arr[c, off + np.arange(n), dloc] = 1.0
                off += int(T[w, s]) * 128
    del per

    def wrap16(a):  # logical i -> [i%16 rep x8, i//16]
        n = a.shape[-1]
        return np.tile(a.reshape(NCORES, n // 16, 16).transpose(0, 2, 1), (1, 8, 1))

    def wrap128(a):  # [C, S, 128] -> [C, 128, S/128, 128] tile-major
        S = a.shape[1]
        return np.ascontiguousarray(
            a.reshape(NCORES, S // 128, 128, 128).transpose(0, 2, 1, 3))

    idxA_w = wrap16(idxA).astype(np.int16)
    idxB_w = wrap16(idxB).astype(np.int16)
    ohA_w = wrap128(ohA).astype(ml_dtypes.float8_e4m3)
    ohB_w = wrap128(ohB).astype(ml_dtypes.float8_e4m3)

    # per-core x shards in T-layout + graph mask
    xo = np.zeros((NCORES, 13, NPAD), np.float32)
    xa = np.zeros((NCORES, 3, NPAD), np.float32)
    xr = np.zeros((NCORES, 3, NPAD), np.float32)
    gmask = np.full((NCORES, 1, NPAD), -1e30, np.float32)
    others = np.concatenate([x[:, :9], x[:, 15:19]], axis=1)  # [N,13]
    for c in range(NCORES):
        sel = np.nonzero(core_of == c)[0]
        cols = col_of[sel]
        xo[c][:, cols] = others[sel].T
        xa[c][:, cols] = x[sel, 9:12].T
        xr[c][:, cols] = x[sel, 12:15].T
        gmask[c, 0, cols] = 0.0

    layout = dict(GP=GP, NPAD=NPAD, NW=NW, SA=SA, SB=SB_, T=T, SPLIT=SPLIT,
                  nta_pad=nta_pad, ntb_pad=ntb_pad)
    shards = dict(xo=xo, xa=xa, xr=xr, gmask=gmask,
                  idxA=idxA_w, idxB=idxB_w, ohA=ohA_w, ohB=ohB_w,
                  ohsel=np.ascontiguousarray(
                      ohsel.transpose(0, 1, 2, 3)).astype(ml_dtypes.float8_e4m3),
                  ohagg=np.ascontiguousarray(
                      ohagg.transpose(0, 1, 2, 3)).astype(ml_dtypes.float8_e4m3))
    return layout, shards


def _prep_weights(P):
    """Transpose / fold weights into the layouts the kernel consumes."""
    w = {}

    def f32(a):
        return np.ascontiguousarray(np.asarray(a, np.float32))

    for p in ("axis", "org"):
        W1, b1 = f32(P[p + "_W1"]), f32(P[p + "_b1"])
        g, be = f32(P[p + "_g"]), f32(P[p + "_be"])
        W2, b2 = f32(P[p + "_W2"]), f32(P[p + "_b2"])
        w[p + "W1T"] = f32(W1.T)                       # [3,16]
        w[p + "b1"] = b1[:, None]                      # [16,1]
        w[p + "W2Tf"] = f32((W2 * g[None, :]).T)       # [16,16]
        w[p + "b2f"] = (W2 @ be + b2)[:, None]         # [16,1]
    ninW = f32(P["nin_W"])  # [128,45]
    w["ninWoT"] = f32(ninW[:, :13].T)   # [13,128]
    w["ninWaT"] = f32(ninW[:, 13:29].T)  # [16,128]
    w["ninWrT"] = f32(ninW[:, 29:45].T)  # [16,128]
    w["ninb"] = f32(P["nin_b"])[:, None]  # [128,1]
    for l in range(N_LAYERS):
        W1, b1 = f32(P["conv_W1"][l]), f32(P["conv_b1"][l])
        g, be = f32(P["conv_g"][l]), f32(P["conv_be"][l])
        W2, b2 = f32(P["conv_W2"][l]), f32(P["conv_b2"][l])
        w[f"W1T{l}"] = f32(W1.T)
        w[f"b1{l}"] = b1[:, None]
        w[f"W2Tf{l}"] = f32((W2 * g[None, :]).T)
        w[f"b2f{l}"] = (W2 @ be + b2)[:, None]
        w[f"lng{l}"] = f32(P["ln_g"][l])[:, None]
        w[f"lnb{l}"] = f32(P["ln_b"][l])[:, None]
        w[f"epsv{l}"] = np.full((128, 1), 1.0 + np.float32(P["eps"][l]), np.float32)
    for p, pre in (("gate", "g"), ("head", "h")):
        W1, b1 = f32(P[p + "_W1"]), f32(P[p + "_b1"])
        g, be = f32(P[p + "_g"]), f32(P[p + "_be"])
        W2, b2 = f32(P[p + "_W2"]), f32(P[p + "_b2"])
        w[pre + "W1T"] = f32(W1.T)                     # [128,64]
        w[pre + "b1"] = b1[:, None]                    # [64,1]
        w[pre + "W2Tf"] = f32((W2 * g[None, :]).T)     # [64,1]
        w[pre + "b2f"] = (W2 @ be + b2)[:, None]       # [1,1]
    return w


# column layout for the packed weight/constant tensor (shared host/builder)
def _wpack_layout():
    cols = {}
    off = 0

    def add(name, rows, ncols):
        nonlocal off
        cols[name] = (off, rows, ncols)
        off += ncols

    for p in ("axis", "org"):
        add(p + "W1T", 3, 16)
        add(p + "b1", 16, 1)
        add(p + "W2Tf", 16, 16)
        add(p + "b2f", 16, 1)
    add("ninWoT", 13, 128)
    add("ninWaT", 16, 128)
    add("ninWrT", 16, 128)
    add("ninb", 128, 1)
    for l in range(N_LAYERS):
        add(f"W1T{l}", 128, 128)
        add(f"b1{l}", 128, 1)
        add(f"W2Tf{l}", 128, 128)
        add(f"b2f{l}", 128, 1)
        add(f"lng{l}", 128, 1)
        add(f"lnb{l}", 128, 1)
        add(f"epsv{l}", 128, 1)
    for pre in ("g", "h"):
        add(pre + "W1T", 128, 64)
        add(pre + "b1", 64, 1)
        add(pre + "W2Tf", 64, 1)
        add(pre + "b2f", 1, 1)
    add("ident", 128, 128)
    add("inv_c1", 128, 1)
    add("inv16", 16, 1)
    add("inv64", 64, 1)
    add("ones_1r", 1, 128)
    add("epsc", 1, 1)
    return cols, off


def _pack_weights(P):
    w = _prep_weights(P)
    w["ident"] = np.eye(128, dtype=np.float32)
    w["inv_c1"] = np.full((128, 1), 1.0 / 128.0, np.float32)
    w["inv16"] = np.full((16, 1), 1.0 / 16.0, np.float32)
    w["inv64"] = np.full((64, 1), 1.0 / 64.0, np.float32)
    w["ones_1r"] = np.ones((1, 128), np.float32)
    w["epsc"] = np.full((1, 1), LN_EPS, np.float32)
    cols, total = _wpack_layout()
    pack = np.zeros((128, total), np.float32)
    for name, (off, rows, ncols) in cols.items():
        a = np.asarray(w[name], np.float32)
        assert a.shape == (rows, ncols), (name, a.shape, rows, ncols)
        pack[:rows, off:off + ncols] = a
    return pack


# ----------------------------------------------------------------------------
# Bass kernel builder
# ----------------------------------------------------------------------------

def _lnT(nc, pools, z_out, y, P, eps=None):
    """LayerNorm over the partition dim (P partitions, n cols), T-layout.
    z_out (sbuf) = (y - mean) * rsqrt(var + eps). y is sbuf [P, n].
    Cross-partition sums/broadcasts run as bf16 matmuls (4x PE rate)."""
    n = y.shape[-1]
    psb = pools["psB"]
    pss = pools["psS"]
    wtile = pools["wtile"]
    stile = pools["stile"]
    ones_c1b = pools["ones_c1b"]
    ones_1rb = pools["ones_1rb"]
    yb = wtile(P, n, BF16)
    nc.scalar.activation(yb[:], y[:], AF.Copy)
    mu = pss.tile([1, n], F32, tag="psS", name="mu")
    nc.tensor.matmul(mu[:], ones_c1b[:P, :], yb[:])            # [1,n] mean
    mu_sb = stile(n, BF16)
    nc.scalar.activation(mu_sb[:], mu[:], AF.Copy)
    mub = psb.tile([128, n], F32, tag="psB", name="mub")
    nc.tensor.matmul(mub[:P, :], ones_1rb[:1, :P], mu_sb[:])   # bcast [P,n]
    d = wtile(P, n)
    nc.vector.tensor_tensor(out=d[:], in0=y[:], in1=mub[:P, :],
                            op=mybir.AluOpType.subtract)
    sq = wtile(P, n, BF16)
    nc.scalar.activation(sq[:], d[:], AF.Square)
    v = pss.tile([1, n], F32, tag="psS", name="v")
    nc.tensor.matmul(v[:], ones_c1b[:P, :], sq[:])             # [1,n] var
    sd = stile(n)
    nc.scalar.activation(sd[:], v[:], AF.Sqrt, bias=pools["epsc"])
    ri_f = stile(n)
    nc.vector.reciprocal_approx_fast(ri_f[:], sd[:])
    ri = stile(n, BF16)
    nc.scalar.activation(ri[:], ri_f[:], AF.Copy)
    rb = psb.tile([128, n], F32, tag="psB", name="rb")
    nc.tensor.matmul(rb[:P, :], ones_1rb[:1, :P], ri[:])       # bcast
    nc.vector.tensor_tensor(out=z_out[:], in0=d[:], in1=rb[:P, :],
                            op=mybir.AluOpType.mult)


def build_kernel(layout):
    GP, NPAD, NW = layout["GP"], layout["NPAD"], layout["NW"]
    SA, SB_ = layout["SA"], layout["SB"]
    T = layout["T"]
    TPC = CH // 128
    TROWS = NCORES * NPAD
    SLAB = 512
    NSLAB = NPAD // SLAB
    WPS = SLAB // 128  # windows (and blocks) per slab
    NTB = NCORES * T2  # sel tiles per block
    REG = W64 * K      # rows per (d, b) region of G
    GROWS = NCORES * W64 * REG
    acum = np.concatenate([[0], np.cumsum(T[:, 0])]).astype(int)
    bcum = np.concatenate([[0], np.cumsum(T[:, 1])]).astype(int)

    nc = bacc.Bacc("TRN2", target_bir_lowering=False, debug=False,
                   num_devices=NCORES, dynamic_dma_scratch_size=49152,
                   num_swdge_queues=NQUEUES)

    def param(name, shape, dtype=F32):
        return nc.declare_dram_parameter(name, list(shape), dtype, isOutput=False)

    xo = param("xo", [13, NPAD])
    xa = param("xa", [3, NPAD])
    xr = param("xr", [3, NPAD])
    gmask = param("gmask", [1, NPAD])
    idxA = param("idxA", [128, SA // 16], mybir.dt.int16)
    idxB = param("idxB", [128, SB_ // 16], mybir.dt.int16)
    ohA = param("ohA", [128, SA // 128, 128], FP8)
    ohB = param("ohB", [128, SB_ // 128, 128], FP8)
    ohsel = param("ohsel", [128, NW * NTB, 128], FP8)
    ohagg = param("ohagg", [128, NW * 4 * K, 128], FP8)
    wcols, wtot = _wpack_layout()
    wpack = param("wpack", [128, wtot])
    out_ext = nc.declare_dram_parameter("out", [1, GPC], F32, isOutput=True)

    from contextlib import ExitStack
    with tile.TileContext(nc) as tc, ExitStack() as es:
            ep = es.enter_context
            constp = ep(tc.tile_pool(name="const", bufs=1))
            persist = ep(tc.tile_pool(name="persist", bufs=1))
            work = ep(tc.tile_pool(name="work", bufs=6))
            wxp = ep(tc.tile_pool(name="wxp", bufs=4))
            wsp = ep(tc.tile_pool(name="wsp", bufs=4))
            gatA = ep(tc.tile_pool(name="gatA", bufs=2))
            gatB = ep(tc.tile_pool(name="gatB", bufs=2))
            ohpA = ep(tc.tile_pool(name="ohpA", bufs=2))
            ohpB = ep(tc.tile_pool(name="ohpB", bufs=2))
            ohselp = ep(tc.tile_pool(name="ohselp", bufs=2))
            ohaggp = ep(tc.tile_pool(name="ohaggp", bufs=2))
            stgp = ep(tc.tile_pool(name="stgp", bufs=2))
            rwp = ep(tc.tile_pool(name="rwp", bufs=2))
            psB = ep(tc.tile_pool(name="psB", bufs=3, space="PSUM"))
            psS = ep(tc.tile_pool(name="psS", bufs=3, space="PSUM"))
            ps128 = ep(tc.tile_pool(name="ps128", bufs=2, space="PSUM"))
            dram = ep(tc.tile_pool(name="dram", bufs=1, space="DRAM"))
            def wtile(rows, ncols, dt=F32):
                return work.tile([128, 1024], dt, tag="w",
                                 name="wt")[:rows, :ncols]

            def stile(ncols, dt=F32):
                return wsp.tile([1, 1024], dt, tag="ws", name="st")[:, :ncols]

            # packed weights
            wp = constp.tile([128, wtot], F32, name="wp")
            nc.sync.dma_start(out=wp[:], in_=wpack[:])
            W = {}
            for nm, (off, rows, ncols) in wcols.items():
                W[nm] = wp[:rows, off:off + ncols]
            ident = W["ident"]
            ones_1r = W["ones_1r"]
            onesb = constp.tile([128, 132], BF16, name="onesb")
            nc.vector.memset(onesb[:], 0.0)
            nc.vector.tensor_copy(onesb[:, 0:1], W["inv_c1"])
            nc.vector.tensor_copy(onesb[:16, 1:2], W["inv16"])
            nc.vector.tensor_copy(onesb[:64, 2:3], W["inv64"])
            nc.vector.tensor_copy(onesb[:1, 4:132], W["ones_1r"])
            pools = dict(psB=psB, psS=psS, ps128=ps128, work=work,
                         epsc=W["epsc"], stile=stile, wtile=wtile,
                         ones_1rb=onesb[:1, 4:132])

            idxA_sb = persist.tile([128, SA // 16], mybir.dt.int16, name="idxAs")
            nc.sync.dma_start(out=idxA_sb[:], in_=idxA[:])
            idxB_sb = persist.tile([128, SB_ // 16], mybir.dt.int16, name="idxBs")
            nc.sync.dma_start(out=idxB_sb[:], in_=idxB[:])
            hT = persist.tile([128, NPAD], F32, name="hT")
            aggT = persist.tile([128, NPAD], BF16, name="aggT")
            Hrow = persist.tile([128, NW * 128], BF16, name="Hrow")
            G = persist.tile([128, GPC], F32, name="Gpool")  # pooled feats

            shard_bf = [dram.tile([NPAD, HID], BF16, tag=f"shard{l}",
                                  name=f"shard{l}") for l in range(N_LAYERS)]
            tables = [dram.tile([TROWS, HID], BF16, addr_space="Shared",
                                tag=f"table{l}", name=f"table{l}")
                      for l in range(N_LAYERS)]
            Gd = dram.tile([GROWS, HID], BF16, tag="Gd", name="Gd")
            Rd = dram.tile([GROWS, HID], BF16, tag="Rd", name="Rd")

            # rearranged views for scatter-flush and window loads
            def g_block_view(b):
                # [p (128), d (8), (t f) (T2*128)] for block b
                v = Gd[:, :].rearrange("(d b p t) f -> b p d (t f)",
                                       d=NCORES, b=W64, p=128, t=T2)
                return v[b]

            def r_window_view(w):
                # [cb (512), (k f) (K*128)] for window w
                v = Rd[:, :].rearrange("(q cb w k) f -> w q cb (k f)",
                                       q=4, cb=128, w=W64, k=K)
                return v[w]

            # zero-fill dummy-block regions of G once
            zstg = stgp.tile([128, NTB * 128], BF16, tag="stg", name="zstg")
            nc.vector.memset(zstg[:], 0.0)
            for b in range(NW, W64):
                nc.sync.dma_start(out=g_block_view(b), in_=zstg[:, :])

            def write_shard(l, s, src_sb):
                """Transpose hT slab s -> row-major bf16: Hrow + shard DRAM."""
                for kk in range(WPS):
                    b = s * WPS + kk
                    tp = ps128.tile([128, 128], F32, tag="ps128", name="tp")
                    nc.tensor.transpose(tp[:], src_sb[:, kk * 128:(kk + 1) * 128],
                                        ident)
                    bslab = Hrow[:, b * 128:(b + 1) * 128]
                    nc.scalar.activation(bslab, tp[:], AF.Copy)
                # one batched DMA for the whole slab: rows b*128+p
                sv = shard_bf[l][:, :].rearrange(
                    "(b p) f -> p b f", b=NW, p=128)[:, s * WPS:(s + 1) * WPS, :]
                nc.scalar.dma_start(out=sv,
                                    in_=Hrow[:, s * SLAB:(s + 1) * SLAB])

            def select_block(b):
                """Edge-selection matmuls for src block b -> G region flush."""
                ohs = ohselp.tile([128, NTB, 128], FP8, tag="ohsel", name="ohs")
                nc.scalar.dma_start(out=ohs[:],
                                    in_=ohsel[:, b * NTB:(b + 1) * NTB, :])
                stg = stgp.tile([128, NTB * 128], BF16, tag="stg", name="stg")
                hrb = Hrow[:, b * 128:(b + 1) * 128]
                for grp in range(NTB // 4):
                    ps = psB.tile([128, 512], F32, tag="psB", name="selp")
                    for j in range(4):
                        ti = grp * 4 + j
                        nc.tensor.matmul(ps[:, j * 128:(j + 1) * 128],
                                         ohs[:, ti, :], hrb)
                    if grp % 2 == 0:
                        nc.scalar.activation(
                            stg[:, grp * 512:(grp + 1) * 512], ps[:], AF.Copy)
                    else:
                        nc.vector.tensor_copy(
                            stg[:, grp * 512:(grp + 1) * 512], ps[:])
                nc.sync.dma_start(out=g_block_view(b), in_=stg[:, :])

            # =========================== encoder ===========================
            for s in range(NSLAB):
                c0, c1 = s * SLAB, (s + 1) * SLAB
                xo_sb = wxp.tile([13, SLAB], F32, tag="wx", name="xos")
                nc.sync.dma_start(out=xo_sb[:], in_=xo[:, c0:c1])
                xa_sb = wxp.tile([3, SLAB], F32, tag="wx", name="xas")
                nc.sync.dma_start(out=xa_sb[:], in_=xa[:, c0:c1])
                xr_sb = wxp.tile([3, SLAB], F32, tag="wx", name="xrs")
                nc.sync.dma_start(out=xr_sb[:], in_=xr[:, c0:c1])
                zz = {}
                for p, xin in (("axis", xa_sb), ("org", xr_sb)):
                    y1p = psB.tile([128, SLAB], F32, tag="psB", name="y1p")
                    nc.tensor.matmul(y1p[:16, :], W[p + "W1T"], xin[:])
                    y1 = wtile(16, SLAB)
                    nc.scalar.activation(y1[:], y1p[:16, :], AF.Relu,
                                         bias=W[p + "b1"])
                    z1 = wtile(16, SLAB)
                    pools["ones_c1b"] = onesb[:, 1:2]
                    _lnT(nc, pools, z1[:], y1[:], 16)
                    zp = psB.tile([128, SLAB], F32, tag="psB", name="zp")
                    nc.tensor.matmul(zp[:16, :], W[p + "W2Tf"], z1[:])
                    z2 = wtile(16, SLAB)
                    nc.vector.tensor_scalar_add(z2[:], zp[:16, :], W[p + "b2f"])
                    zz[p] = z2
                h0p = psB.tile([128, SLAB], F32, tag="psB", name="h0p")
                nc.tensor.matmul(h0p[:], W["ninWoT"], xo_sb[:],
                                 start=True, stop=False)
                nc.tensor.matmul(h0p[:], W["ninWaT"], zz["axis"][:],
                                 start=False, stop=False)
                nc.tensor.matmul(h0p[:], W["ninWrT"], zz["org"][:],
                                 start=False, stop=True)
                nc.vector.tensor_scalar_add(hT[:, c0:c1], h0p[:], W["ninb"])
                write_shard(0, s, hT[:, c0:c1])
                for kk in range(WPS):
                    select_block(s * WPS + kk)

            # table gather (ready first) + exchange for layer 0
            nc.gpsimd.collective_compute(
                "AllGather", mybir.AluOpType.bypass,
                replica_groups=[list(range(NCORES))],
                ins=[shard_bf[0].opt()], outs=[tables[0].opt()],
            )
            nc.gpsimd.collective_compute(
                "AllToAll", mybir.AluOpType.bypass,
                replica_groups=[list(range(NCORES))],
                ins=[Gd[:, :]], outs=[Rd[:, :]],
            )

            # =========================== GIN layers ===========================
            qsel = [0]
            for l in range(N_LAYERS):
                side_cfg = (
                    (idxA_sb, ohA, tables[l][0:SPLIT, :], gatA, ohpA),
                    (idxB_sb, ohB, tables[l][SPLIT:TROWS, :], gatB, ohpB),
                )
                chunk_tiles = [{}, {}]

                def get_chunk(sd, c):
                    if c not in chunk_tiles[sd]:
                        idx_sb, oh_dram, tbl_s, gp, op = side_cfg[sd]
                        g = gp.tile([128, TPC, HID], BF16, tag=f"gat{sd}",
                                    name=f"g{sd}")
                        nc.gpsimd.dma_gather(
                            out_ap=g[:],
                            in_ap=tbl_s[:, :],
                            idxs_ap=idx_sb[:, c * (CH // 16):(c + 1) * (CH // 16)],
                            num_idxs=CH,
                            num_idxs_reg=CH,
                            elem_size=HID,
                            queue_num=qsel[0] % NQUEUES,
                        )
                        qsel[0] += 1
                        oh = op.tile([128, TPC, 128], FP8, tag=f"oh{sd}",
                                     name=f"oh{sd}")
                        nc.sync.dma_start(
                            out=oh[:],
                            in_=oh_dram[:, c * TPC:(c + 1) * TPC, :])
                        chunk_tiles[sd][c] = (g, oh)
                    return chunk_tiles[sd][c]

                for s in range(NSLAB):
                    for w in range(s * WPS, (s + 1) * WPS):
                        # structured cells
                        rw = rwp.tile([128, 4 * K * 128], BF16, tag="rw",
                                      name="rw")
                        nc.sync.dma_start(out=rw[:, :], in_=r_window_view(w))
                        oha = ohaggp.tile([128, 4 * K, 128], FP8, tag="ohagg",
                                          name="oha")
                        nc.scalar.dma_start(
                            out=oha[:],
                            in_=ohagg[:, w * 4 * K:(w + 1) * 4 * K, :])
                        seq = ([(0, acum[w] + j) for j in range(T[w, 0])]
                               + [(1, bcum[w] + j) for j in range(T[w, 1])])
                        psag = ps128.tile([128, 128], F32, tag="ps128",
                                          name="psag")
                        nmm = 4 * K + len(seq)
                        i = 0
                        for jk in range(4 * K):
                            nc.tensor.matmul(
                                psag[:],
                                rw[:, jk * 128:(jk + 1) * 128],
                                oha[:, jk, :],
                                start=(i == 0), stop=(i == nmm - 1))
                            i += 1
                        for (sd, gt) in seq:
                            g, oh = get_chunk(sd, gt // TPC)
                            t = gt % TPC
                            nc.tensor.matmul(psag[:], g[:, t, :], oh[:, t, :],
                                             start=(i == 0),
                                             stop=(i == nmm - 1))
                            i += 1
                        nc.vector.tensor_copy(aggT[:, w * 128:(w + 1) * 128],
                                              psag[:])

                    c0, c1 = s * SLAB, (s + 1) * SLAB
                    m = wtile(128, SLAB)
                    nc.vector.scalar_tensor_tensor(
                        out=m[:], in0=hT[:, c0:c1], scalar=W[f"epsv{l}"],
                        in1=aggT[:, c0:c1], op0=mybir.AluOpType.mult,
                        op1=mybir.AluOpType.add)
                    y1p = psB.tile([128, SLAB], F32, tag="psB", name="y1p")
                    nc.tensor.matmul(y1p[:], W[f"W1T{l}"], m[:])
                    y1 = wtile(128, SLAB)
                    nc.scalar.activation(y1[:], y1p[:], AF.Relu, bias=W[f"b1{l}"])
                    z1 = wtile(128, SLAB)
                    pools["ones_c1b"] = onesb[:, 0:1]
                    _lnT(nc, pools, z1[:], y1[:], 128)
                    y2p = psB.tile([128, SLAB], F32, tag="psB", name="y2p")
                    nc.tensor.matmul(y2p[:], W[f"W2Tf{l}"], z1[:])
                    r2 = wtile(128, SLAB)
                    nc.scalar.activation(r2[:], y2p[:], AF.Relu, bias=W[f"b2f{l}"])
                    z2 = wtile(128, SLAB)
                    pools["ones_c1b"] = onesb[:, 0:1]
                    _lnT(nc, pools, z2[:], r2[:], 128)
                    zs = wtile(128, SLAB)
                    nc.vector.tensor_scalar(out=zs[:], in0=z2[:],
                                            scalar1=W[f"lng{l}"],
                                            op0=mybir.AluOpType.mult,
                                            scalar2=W[f"lnb{l}"],
                                            op1=mybir.AluOpType.add)
                    nc.vector.tensor_tensor(out=hT[:, c0:c1], in0=zs[:],
                                            in1=hT[:, c0:c1],
                                            op=mybir.AluOpType.add)
                    if l + 1 < N_LAYERS:
                        write_shard(l + 1, s, hT[:, c0:c1])
                        for kk in range(WPS):
                            select_block(s * WPS + kk)

                if l + 1 < N_LAYERS:
                    nc.gpsimd.collective_compute(
                        "AllGather", mybir.AluOpType.bypass,
                        replica_groups=[list(range(NCORES))],
                        ins=[shard_bf[l + 1].opt()], outs=[tables[l + 1].opt()],
                    )
                    nc.gpsimd.collective_compute(
                        "AllToAll", mybir.AluOpType.bypass,
                        replica_groups=[list(range(NCORES))],
                        ins=[Gd[:, :]], outs=[Rd[:, :]],
                    )

            # ===================== pooling + head =====================
            gate_d = dram.tile([1, NPAD], F32, tag="gate_d", name="gate_d")
            for s in range(NSLAB):
                c0, c1 = s * SLAB, (s + 1) * SLAB
                y1p = psB.tile([128, SLAB], F32, tag="psB", name="y1p")
                nc.tensor.matmul(y1p[:64, :], W["gW1T"], hT[:, c0:c1])
                y1 = wtile(64, SLAB)
                nc.scalar.activation(y1[:], y1p[:64, :], AF.Relu, bias=W["gb1"])
                z1 = wtile(64, SLAB)
                pools["ones_c1b"] = onesb[:, 2:3]
                _lnT(nc, pools, z1[:], y1[:], 64)
                gp = psS.tile([1, SLAB], F32, tag="psS", name="gp")
                nc.tensor.matmul(gp[:], W["gW2Tf"], z1[:])
                gsb = stile(SLAB)
                nc.vector.tensor_scalar_add(gsb[:], gp[:], W["gb2f"])
                gm_sb = stile(SLAB)
                nc.sync.dma_start(out=gm_sb[:], in_=gmask[:, c0:c1])
                gfin = stile(SLAB)
                nc.vector.tensor_tensor(out=gfin[:], in0=gsb[:], in1=gm_sb[:],
                                        op=mybir.AluOpType.add)
                nc.sync.dma_start(out=gate_d[:, c0:c1], in_=gfin[:])

            for gi in range(GPC):
                c0, c1 = gi * GP, (gi + 1) * GP
                gate_sb = stile(GP)
                nc.sync.dma_start(out=gate_sb[:], in_=gate_d[:, c0:c1])
                gmax = stile(1)
                nc.vector.tensor_reduce(out=gmax[:], in_=gate_sb[:],
                                        axis=mybir.AxisListType.X,
                                        op=mybir.AluOpType.max)
                negmax = stile(1)
                nc.vector.tensor_scalar_mul(negmax[:], gmax[:], -1.0)
                e = stile(GP)
                nc.scalar.activation(e[:], gate_sb[:], AF.Exp, bias=negmax[:])
                den = stile(1)
                nc.vector.tensor_reduce(out=den[:], in_=e[:],
                                        axis=mybir.AxisListType.X,
                                        op=mybir.AluOpType.add)
                rden = stile(1)
                nc.vector.reciprocal(rden[:], den[:])
                attn = stile(GP)
                nc.vector.tensor_scalar_mul(attn[:], e[:], rden[:])
                half = GP // 2
                gacc = wtile(128, 2)
                for hf in range(2):
                    h0, h1 = c0 + hf * half, c0 + (hf + 1) * half
                    ab = psB.tile([128, half], F32, tag="psB", name="ab")
                    nc.tensor.matmul(ab[:], ones_1r,
                                     attn[:, hf * half:(hf + 1) * half])
                    wh = wtile(128, half)
                    nc.vector.tensor_tensor(out=wh[:], in0=hT[:, h0:h1],
                                            in1=ab[:], op=mybir.AluOpType.mult)
                    nc.vector.tensor_reduce(out=gacc[:, hf:hf + 1], in_=wh[:],
                                            axis=mybir.AxisListType.X,
                                            op=mybir.AluOpType.add)
                nc.vector.tensor_reduce(out=G[:, gi:gi + 1], in_=gacc[:],
                                        axis=mybir.AxisListType.X,
                                        op=mybir.AluOpType.add)

            y1p = psB.tile([128, GPC], F32, tag="psB", name="y1ph")
            nc.tensor.matmul(y1p[:64, :], W["hW1T"], G[:])
            y1 = wtile(64, GPC)
            nc.scalar.activation(y1[:], y1p[:64, :], AF.Relu, bias=W["hb1"])
            z1 = wtile(64, GPC)
            pools["ones_c1b"] = onesb[:, 2:3]
            _lnT(nc, pools, z1[:], y1[:], 64)
            lp = psS.tile([1, GPC], F32, tag="psS", name="lp")
            nc.tensor.matmul(lp[:], W["hW2Tf"], z1[:])
            logit = stile(GPC)
            nc.vector.tensor_scalar_add(logit[:], lp[:], W["hb2f"])
            nc.sync.dma_start(out=out_ext[:], in_=logit[:])

    nc.compile()
    return nc


# ----------------------------------------------------------------------------
# Public entry point
# ----------------------------------------------------------------------------

_CACHE = {}


def kernel(**inputs):
    x = inputs["x"]
    edge_index = inputs["edge_index"]
    batch = inputs["batch"]

    layout, shards = _prep(x, edge_index, batch)
    wpack = _pack_weights(inputs)

    key = (layout["GP"], layout["SA"], layout["SB"],
           layout["T"].tobytes())
    nc = _CACHE.get(key)
    if nc is None:
        nc = build_kernel(layout)
        _CACHE[key] = nc

    in_maps = []
    for c in range(NCORES):
        m = {
            "xo": shards["xo"][c], "xa": shards["xa"][c], "xr": shards["xr"][c],
            "gmask": shards["gmask"][c],
            "idxA": shards["idxA"][c], "idxB": shards["idxB"][c],
            "ohA": shards["ohA"][c], "ohB": shards["ohB"][c],
            "ohsel": shards["ohsel"][c], "ohagg": shards["ohagg"][c],
            "wpack": wpack,
        }
        in_maps.append(m)

    import os
    trace = os.environ.get("GNN_KERNEL_TRACE", "1") == "1"
    try:
        res = run_bass_kernel_spmd(nc, in_maps, core_ids=list(range(NCORES)),
                                   trace=trace)
    except Exception:
        if not trace:
            raise
        res = run_bass_kernel_spmd(nc, in_maps, core_ids=list(range(NCORES)),
                                   trace=False)
    global LAST_EXEC_NS
    LAST_EXEC_NS = res.exec_time_ns
    out = np.concatenate([np.asarray(res.results[c]["out"]).reshape(-1)
                          for c in range(NCORES)])
    return out.astype(np.float32)


LAST_EXEC_NS = None


if __name__ == "__main__":
    sys.path.insert(0, "/root/problem")
    import reference
    inp = reference.setup_inputs()
    got = kernel(**{k: np.asarray(v) for k, v in inp.items()})
    exp = np.asarray(reference.reference(**inp))
    err = np.linalg.norm(got - exp) / np.linalg.norm(exp)
    print("Relative error:", err)
